# revision 1
# baseline (speedup 1.0000x reference)
"""Trainium2 Bass kernel for MinimalLightningIndexer.

out[b,t,s] = relu((x@Wq)[b,t] . (x@Wk)[b,s]) * (x@Ww)[b,t]

Sharding: 8 cores = 4 batches x 2 query-halves. Each core computes the
[2048, 4096] score block for its (batch, t-half). The host feeds each
core x[b].T (d-major, contiguous) with the core's own t-half tokens
permuted to the front, so one SPMD program serves all cores; the host
un-permutes score columns when assembling the full output.

Per-core device program:
  - load x.T slabs [2048d x 512tok] (4 MB DMAs, natural layout)
  - PE: kT[16,512] per token chunk (all 8), qT/wT[17,512] (own 4 chunks),
    f32 matmuls accumulating over 16 d-chunks of 128
  - one SBUF->SBUF DMA transposes wT[1,2048] -> w_col[128,16]
  - scores: matmul qT_tile.T @ kT chunk (K=16, N=512) -> PSUM,
    ScalarE relu PSUM->SBUF, VectorE per-partition gate multiply,
    1 MB output DMAs
"""

import sys

if "/opt/trn_rl_repo" not in sys.path:
    sys.path.insert(0, "/opt/trn_rl_repo")

import numpy as np

import concourse.bacc as bacc
import concourse.bass as bass
import concourse.mybir as mybir
import concourse.tile as tile
from concourse.bass_utils import run_bass_kernel_spmd

B, S, D = 4, 4096, 2048
IDX = 16
N_CORES = 8
T = S // 2          # query tokens per core
DC = D // 128       # 16 d-chunks
SC = S // 512       # 8 token chunks
TC = T // 512       # 4 own token chunks
TT = T // 128       # 16 t-tiles

_CACHE = {}


def _build_nc():
    if "nc" in _CACHE:
        return _CACHE["nc"]
    f32 = mybir.dt.float32
    bf16 = mybir.dt.bfloat16
    nc = bacc.Bacc("TRN2", target_bir_lowering=False, debug=False,
                   num_devices=N_CORES)
    xt = nc.dram_tensor("xt", [D, S], bf16, kind="ExternalInput").ap()
    wk = nc.dram_tensor("wk", [D, IDX], bf16, kind="ExternalInput").ap()
    wqw = nc.dram_tensor("wqw", [D, IDX + 1], bf16, kind="ExternalInput").ap()
    o = nc.dram_tensor("o", [T, S], bf16, kind="ExternalOutput").ap()

    with tile.TileContext(nc) as tc:
        with (
            tc.tile_pool(name="const", bufs=1) as cpool,
            tc.tile_pool(name="slab", bufs=3) as slab_pool,
            tc.tile_pool(name="osb", bufs=4) as out_pool,
            tc.tile_pool(name="pk", bufs=2, space="PSUM") as pk_pool,
            tc.tile_pool(name="pqw", bufs=2, space="PSUM") as pqw_pool,
            tc.tile_pool(name="ps", bufs=4, space="PSUM") as ps_pool,
        ):
            # --- persistent small tensors ---
            wk_sb = cpool.tile([128, DC * IDX], bf16, tag="wk_sb")
            nc.sync.dma_start(
                out=wk_sb[:],
                in_=wk.rearrange("(kd p) i -> p kd i", p=128),
            )
            wqw_sb = cpool.tile([128, DC * (IDX + 1)], bf16, tag="wqw_sb")
            nc.sync.dma_start(
                out=wqw_sb[:],
                in_=wqw.rearrange("(kd p) i -> p kd i", p=128),
            )
            kt_sb = cpool.tile([IDX, S], bf16, tag="kt_sb")
            qw_sb = cpool.tile([IDX + 1, T], bf16, tag="qw_sb")
            qwf_sb = cpool.tile([IDX + 1, T], f32, tag="qwf_sb")
            w_col = cpool.tile([128, TT], f32, tag="w_col")

            # --- projections per 512-token chunk ---
            for j in range(SC):
                slab = slab_pool.tile([128, DC * 512], bf16, tag="slab")
                nc.sync.dma_start(
                    out=slab[:],
                    in_=xt[:, j * 512:(j + 1) * 512].rearrange(
                        "(kd p) s -> p kd s", p=128),
                )
                slab_v = slab[:].rearrange("p (kd t) -> p kd t", kd=DC)

                psk = pk_pool.tile([IDX, 512], f32, tag="psk")
                for kd in range(DC):
                    nc.tensor.matmul(
                        psk[:],
                        wk_sb[:, kd * IDX:(kd + 1) * IDX],
                        slab_v[:, kd, :],
                        start=(kd == 0), stop=(kd == DC - 1),
                    )
                nc.vector.tensor_copy(kt_sb[:, j * 512:(j + 1) * 512], psk[:])

                if j < TC:
                    psqw = pqw_pool.tile([IDX + 1, 512], f32, tag="psqw")
                    for kd in range(DC):
                        nc.tensor.matmul(
                            psqw[:],
                            wqw_sb[:, kd * (IDX + 1):(kd + 1) * (IDX + 1)],
                            slab_v[:, kd, :],
                            start=(kd == 0), stop=(kd == DC - 1),
                        )
                    nc.vector.tensor_copy(
                        qw_sb[:, j * 512:(j + 1) * 512], psqw[:])
                    nc.vector.tensor_copy(
                        qwf_sb[:, j * 512:(j + 1) * 512], psqw[:])

            # --- transpose gate row wT[1, T] -> w_col[128, TT] ---
            for i in range(TT):
                nc.sync.dma_start(
                    out=w_col[:, i:i + 1],
                    in_=qwf_sb[IDX:IDX + 1, i * 128:(i + 1) * 128],
                )

            # --- scores ---
            for i in range(TT):
                for jq in range(2):
                    osb = out_pool.tile([128, 2048], bf16, tag="osb")
                    for jj in range(4):
                        j = jq * 4 + jj
                        pss = ps_pool.tile([128, 512], f32, tag="pss")
                        nc.tensor.matmul(
                            pss[:],
                            qw_sb[0:IDX, i * 128:(i + 1) * 128],
                            kt_sb[:, j * 512:(j + 1) * 512],
                            start=True, stop=True,
                        )
                        nc.scalar.activation(
                            osb[:, jj * 512:(jj + 1) * 512], pss[:],
                            mybir.ActivationFunctionType.Relu,
                        )
                        nc.vector.tensor_scalar_mul(
                            out=osb[:, jj * 512:(jj + 1) * 512],
                            in0=osb[:, jj * 512:(jj + 1) * 512],
                            scalar1=w_col[:, i:i + 1],
                        )
                    nc.sync.dma_start(
                        out=o[i * 128:(i + 1) * 128,
                              jq * 2048:(jq + 1) * 2048],
                        in_=osb[:],
                    )
    nc.compile()
    _CACHE["nc"] = nc
    return nc


def _make_in_maps(x, Wq, Wk, Ww):
    import ml_dtypes
    bf = ml_dtypes.bfloat16
    wqw = np.ascontiguousarray(
        np.concatenate([Wq, Ww], axis=1)).astype(bf)
    wk = np.ascontiguousarray(Wk).astype(bf)
    xbf = x.astype(bf)
    in_maps = []
    for c in range(N_CORES):
        b, h = c // 2, c % 2
        own = xbf[b, h * T:(h + 1) * T, :]
        oth = xbf[b, (1 - h) * T:(2 - h) * T, :]
        xt = np.ascontiguousarray(np.concatenate([own, oth], axis=0).T)
        in_maps.append({"xt": xt, "wk": wk, "wqw": wqw})
    return in_maps


def _assemble(results):
    out = np.empty((B, S, S), dtype=np.float32)
    for c in range(N_CORES):
        b, h = c // 2, c % 2
        oc = np.asarray(results[c]["o"], dtype=np.float32)
        if h == 1:
            oc = np.concatenate([oc[:, T:], oc[:, :T]], axis=1)
        out[b, h * T:(h + 1) * T, :] = oc
    return out


def kernel(x, Wq, Wk, Ww, _trace_kwargs=None):
    nc = _build_nc()
    in_maps = _make_in_maps(np.asarray(x, dtype=np.float32),
                            np.asarray(Wq, dtype=np.float32),
                            np.asarray(Wk, dtype=np.float32),
                            np.asarray(Ww, dtype=np.float32))
    kw = _trace_kwargs or {}
    res = run_bass_kernel_spmd(nc, in_maps, list(range(N_CORES)), **kw)
    out = _assemble(res.results)
    if _trace_kwargs is not None:
        return out, res
    return out



# revision 20
# speedup vs baseline: 1.1380x; 1.1380x over previous
"""Trainium2 Bass kernel for MinimalLightningIndexer.

out[b,t,s] = relu((x@Wq)[b,t] . (x@Wk)[b,s]) * (x@Ww)[b,t]

Sharding: 8 cores = 4 batches x 2 token-halves. Each core receives ONLY
its own half of x[b] (transposed, 8.4 MB bf16), computes [k|q|w]
projections for its 2048 tokens in one fused PE pass, and the 16-dim key
rows are exchanged with the sibling core via paired AllGather
collectives (2 stages x 32 KB) so each core scores its 2048 queries
against all 4096 keys. Output [2048, 4096] bf16 is written as half-rows
in two phases keyed to the exchange stages; the host reassembles with 4
block copies per core.

Per-core device program:
  - 4 input slabs [128p, 16kd, 512t] (16 KB descriptors)
  - projections: 16 f32-accum matmuls per slab, lhsT = wkqw [128, 33]
  - exchange: k rows -> DRAM bounce -> AllGather({2c,2c+1}) -> kboth
  - scores: qT [16,128] x kboth chunk [16,512] -> PSUM [128,1024] groups
  - postproc: 3/4 ACT relu + DVE bf16 gate, 1/4 DVE fused (max0, mult w)
  - output: half-row [128, 2048] DMAs (4 KB descriptors)
"""

import os
import sys

if "/opt/trn_rl_repo" not in sys.path:
    sys.path.insert(0, "/opt/trn_rl_repo")

import numpy as np

import concourse.bacc as bacc
import concourse.bass as bass
import concourse.mybir as mybir
import concourse.tile as tile
from concourse.bass_utils import run_bass_kernel_spmd

B, S, D = 4, 4096, 2048
IDX = 16
N_CORES = 8
T = S // 2           # own tokens per core
DC = D // 128        # 16 d-chunks
NG = 4               # projection groups (512 tokens each)
W33 = 2 * IDX + 1    # [k | q | w] projection width

USE_CC = os.environ.get("K_USE_CC", "1") == "1"    # AllGather k-exchange
PS_BANKS = int(os.environ.get("K_PS_BANKS", "2"))  # PSUM banks per group
FUSED = os.environ.get("K_FUSED", "1") == "1"      # DVE fused relu+gate

_CACHE = {}


def _build_nc():
    if "nc" in _CACHE:
        return _CACHE["nc"]
    f32 = mybir.dt.float32
    bf16 = mybir.dt.bfloat16
    nc = bacc.Bacc("TRN2", target_bir_lowering=False, debug=False,
                   num_devices=N_CORES)

    n_slabs = NG if USE_CC else 2 * NG
    xh = nc.dram_tensor("xh", [n_slabs * 128, DC * 512], bf16,
                        kind="ExternalInput").ap()
    wkqw = nc.dram_tensor("wkqw", [128, DC * W33], bf16,
                          kind="ExternalInput").ap()
    o = nc.dram_tensor("o", [T, S], bf16, kind="ExternalOutput").ap()

    groups = [[2 * i, 2 * i + 1] for i in range(N_CORES // 2)]

    with tile.TileContext(nc) as tc:
        with (
            tc.tile_pool(name="const", bufs=1) as cpool,
            tc.tile_pool(name="slab", bufs=3) as slab_pool,
            tc.tile_pool(name="osb", bufs=6) as out_pool,
            tc.tile_pool(name="pj", bufs=2, space="PSUM") as pj_pool,
            tc.tile_pool(name="ps", bufs=6 // PS_BANKS, space="PSUM") as ps_pool,
            tc.tile_pool(name="dram", bufs=1, space="DRAM") as dpool,
        ):
            # --- persistent small tensors ---
            wkqw_sb = cpool.tile([128, DC * W33], bf16, tag="wkqw_sb")
            nc.sync.dma_start(out=wkqw_sb[:], in_=wkqw)

            s33_sb = cpool.tile([W33, T], bf16, tag="s33_sb")
            qT_sb = cpool.tile([IDX, T], bf16, tag="qT_sb")
            kboth_sb = cpool.tile([IDX, S], bf16, tag="kboth_sb")
            w_colb = cpool.tile([128, T // 128], bf16, tag="w_colb")
            w_col = cpool.tile([128, T // 128], f32, tag="w_col")

            if USE_CC:
                kin = [dpool.tile([IDX, 1024], bf16, name=f"kin{s}",
                                  tag=f"kin{s}") for s in range(2)]
                kg = [dpool.tile([2 * IDX, 1024], bf16, name=f"kg{s}",
                                 tag=f"kg{s}") for s in range(2)]

            # --- input slabs (ACT hwdge queue) ---
            slabs = []
            for s in range(n_slabs):
                slab = slab_pool.tile([128, DC * 512], bf16, tag="slab")
                nc.scalar.dma_start(
                    out=slab[:], in_=xh[s * 128:(s + 1) * 128, :])
                slabs.append(slab)

            # --- projections per 512-token group ---
            for g in range(n_slabs):
                slab_v = slabs[g][:].rearrange("p (kd t) -> p kd t", kd=DC)
                pj = pj_pool.tile([W33, 512], f32, tag="pj")
                for kd in range(DC):
                    nc.tensor.matmul(
                        pj[:],
                        wkqw_sb[:, kd * W33:(kd + 1) * W33],
                        slab_v[:, kd, :],
                        start=(kd == 0), stop=(kd == DC - 1),
                    )
                c0, c1 = g * 512, (g + 1) * 512
                if g < NG:
                    nc.vector.tensor_copy(s33_sb[:, c0:c1], pj[:])
                    # engine reads need 32-aligned partition offsets;
                    # DMAs don't — extract q rows and transposed w by DMA
                    nc.gpsimd.dma_start(
                        out=qT_sb[:, c0:c1],
                        in_=s33_sb[IDX:2 * IDX, c0:c1])
                    # w gate transposed column-by-column (ACT hwdge queue)
                    for gi in range(4):
                        t0 = c0 + gi * 128
                        nc.scalar.dma_start(
                            out=w_colb[:, g * 4 + gi:g * 4 + gi + 1],
                            in_=s33_sb[2 * IDX:W33, t0:t0 + 128],
                        )
                    nc.vector.tensor_copy(
                        w_col[:, g * 4:(g + 1) * 4],
                        w_colb[:, g * 4:(g + 1) * 4])
                    if USE_CC and g % 2 == 1:
                        st = g // 2
                        nc.gpsimd.dma_start(
                            out=kin[st][:],
                            in_=s33_sb[0:IDX, st * 1024:(st + 1) * 1024])
                        nc.gpsimd.collective_compute(
                            "AllGather",
                            mybir.AluOpType.bypass,
                            replica_groups=groups,
                            ins=[kin[st].opt()],
                            outs=[kg[st].opt()],
                        )
                else:
                    # fallback path: other-half keys computed locally
                    nc.vector.tensor_copy(
                        kboth_sb[:, T + c0 - NG * 512:T + c1 - NG * 512],
                        pj[0:IDX, :])

            if USE_CC:
                # stage-0 gathered keys (SP queue, ahead of outputs)
                for r in range(2):
                    nc.sync.dma_start(
                        out=kboth_sb[:, r * 1024:(r + 1) * 1024],
                        in_=kg[0][r * IDX:(r + 1) * IDX, :],
                    )
                # stage-1 gathered keys (gpsimd queue, after collective 1)
                for r in range(2):
                    nc.gpsimd.dma_start(
                        out=kboth_sb[:, 2048 + r * 1024:2048 + (r + 1) * 1024],
                        in_=kg[1][r * IDX:(r + 1) * IDX, :],
                    )
            else:
                for g in range(NG):
                    c0, c1 = g * 512, (g + 1) * 512
                    nc.vector.tensor_copy(
                        kboth_sb[:, c0:c1], kown_sb[:, c0:c1])

            # --- scores + postproc + output, two column phases ---
            TT = T // 128
            gw = PS_BANKS * 512            # postproc group width
            for ph in range(2):
                col0 = ph * 2048
                for i in range(TT):
                    osb = out_pool.tile([128, 2048], bf16, tag="osb")
                    for cc in range(2048 // gw):
                        ps = ps_pool.tile([128, gw], f32, tag="ps")
                        for jj in range(PS_BANKS):
                            j0 = col0 + cc * gw + jj * 512
                            nc.tensor.matmul(
                                ps[:, jj * 512:(jj + 1) * 512],
                                qT_sb[:, i * 128:(i + 1) * 128],
                                kboth_sb[:, j0:j0 + 512],
                                start=True, stop=True,
                            )
                        oslice = osb[:, cc * gw:(cc + 1) * gw]
                        if FUSED and (i * (2048 // gw) + cc) % 4 == 3:
                            # fused relu+gate on DVE
                            nc.vector.tensor_scalar(
                                out=oslice,
                                in0=ps[:],
                                scalar1=0.0,
                                scalar2=w_col[:, i:i + 1],
                                op0=mybir.AluOpType.max,
                                op1=mybir.AluOpType.mult,
                            )
                        else:
                            nc.scalar.activation(
                                oslice, ps[:],
                                mybir.ActivationFunctionType.Relu,
                            )
                            nc.vector.tensor_scalar_mul(
                                out=oslice,
                                in0=oslice,
                                scalar1=w_col[:, i:i + 1],
                            )
                    nc.sync.dma_start(
                        out=o[i * 128:(i + 1) * 128, col0:col0 + 2048],
                        in_=osb[:],
                    )
    nc.compile()
    _CACHE["nc"] = nc
    return nc


def _make_in_maps(x, Wq, Wk, Ww):
    import ml_dtypes
    bf = ml_dtypes.bfloat16
    w33 = np.concatenate([Wk, Wq, Ww], axis=1).astype(bf)       # [D, 33]
    wkqw = np.ascontiguousarray(
        w33.reshape(DC, 128, W33).transpose(1, 0, 2).reshape(128, DC * W33))
    xbf = x.astype(bf)
    n_slabs = NG if USE_CC else 2 * NG
    in_maps = []
    for c in range(N_CORES):
        b, h = c // 2, c % 2
        own = xbf[b, h * T:(h + 1) * T, :]                       # [T, D]
        if USE_CC:
            xt = own.T                                            # [D, T]
        else:
            oth = xbf[b, (1 - h) * T:(2 - h) * T, :]
            xt = np.concatenate([own, oth], axis=0).T             # [D, S]
        ntok = xt.shape[1]
        xs = np.ascontiguousarray(
            xt.reshape(DC, 128, ntok // 512, 512)
            .transpose(2, 1, 0, 3).reshape(n_slabs * 128, DC * 512))
        in_maps.append({"xh": xs, "wkqw": wkqw})
    return in_maps


def _assemble(results):
    out = np.empty((B, S, S), dtype=np.float32)
    for c in range(N_CORES):
        b, h = c // 2, c % 2
        oc = np.asarray(results[c]["o"], dtype=np.float32)
        r0 = h * T
        if USE_CC:
            # columns: [s0 slot0 | s0 slot1 | s1 slot0 | s1 slot1]
            out[b, r0:r0 + T, 0:1024] = oc[:, 0:1024]
            out[b, r0:r0 + T, 2048:3072] = oc[:, 1024:2048]
            out[b, r0:r0 + T, 1024:2048] = oc[:, 2048:3072]
            out[b, r0:r0 + T, 3072:4096] = oc[:, 3072:4096]
        else:
            out[b, r0:r0 + T, h * T:(h + 1) * T] = oc[:, 0:T]
            out[b, r0:r0 + T, (1 - h) * T:(2 - h) * T] = oc[:, T:S]
    return out


def kernel(x, Wq, Wk, Ww, _trace_kwargs=None):
    nc = _build_nc()
    in_maps = _make_in_maps(np.asarray(x, dtype=np.float32),
                            np.asarray(Wq, dtype=np.float32),
                            np.asarray(Wk, dtype=np.float32),
                            np.asarray(Ww, dtype=np.float32))
    kw = _trace_kwargs or {}
    res = run_bass_kernel_spmd(nc, in_maps, list(range(N_CORES)), **kw)
    out = _assemble(res.results)
    if _trace_kwargs is not None:
        return out, res
    return out


# revision 21
# speedup vs baseline: 1.2094x; 1.0627x over previous
"""Trainium2 Bass kernel for MinimalLightningIndexer.

out[b,t,s] = relu((x@Wq)[b,t] . (x@Wk)[b,s]) * (x@Ww)[b,t]

Sharding: 8 cores = 4 batches x 2 token-halves. Each core receives ONLY
its own half of x[b] (transposed, 8.4 MB bf16) and computes [k|q|w]
projections for its 2048 tokens in one fused PE pass. Phase A scores the
2048 own queries against the 2048 LOCAL keys immediately; meanwhile one
pairwise AllGather exchanges the 16-dim key rows (64 KB) with the
sibling core, and phase B scores against the sibling keys (selected from
the gather output by a per-core index-gather DMA, keeping the program
SPMD-uniform). Output [2048, 4096] bf16 = [own cols | sibling cols]; the
host reassembles with 2 block copies per core.

Per-core device program:
  - 4 input slabs [128p, 16kd, 512t] (16 KB descriptors)
  - projections: 16 f32-accum matmuls per slab, lhsT = wkqw [128, 33]
  - exchange: k rows -> DRAM bounce -> AllGather({2c,2c+1}) -> gather
  - scores: qT [16,128] x k chunk [16,512] -> PSUM [128,1024] groups
  - postproc: 3/4 ACT relu + DVE bf16 gate, 1/4 DVE fused (max0, mult w)
  - output: half-row [128, 2048] DMAs (4 KB descriptors)
"""

import os
import sys

if "/opt/trn_rl_repo" not in sys.path:
    sys.path.insert(0, "/opt/trn_rl_repo")

import numpy as np

import concourse.bacc as bacc
import concourse.bass as bass
import concourse.mybir as mybir
import concourse.tile as tile
from concourse.bass_utils import run_bass_kernel_spmd

B, S, D = 4, 4096, 2048
IDX = 16
N_CORES = 8
T = S // 2           # own tokens per core
DC = D // 128        # 16 d-chunks
NG = 4               # projection groups (512 tokens each)
W33 = 2 * IDX + 1    # [k | q | w] projection width

USE_CC = os.environ.get("K_USE_CC", "1") == "1"    # AllGather k-exchange
PS_BANKS = int(os.environ.get("K_PS_BANKS", "2"))  # PSUM banks per group
FUSED = os.environ.get("K_FUSED", "1") == "1"      # DVE fused relu+gate

_CACHE = {}


def _build_nc():
    if "nc" in _CACHE:
        return _CACHE["nc"]
    f32 = mybir.dt.float32
    bf16 = mybir.dt.bfloat16
    i32 = mybir.dt.int32
    nc = bacc.Bacc("TRN2", target_bir_lowering=False, debug=False,
                   num_devices=N_CORES)

    n_slabs = NG if USE_CC else 2 * NG
    xh = nc.dram_tensor("xh", [n_slabs * 128, DC * 512], bf16,
                        kind="ExternalInput").ap()
    wkqw = nc.dram_tensor("wkqw", [128, DC * W33], bf16,
                          kind="ExternalInput").ap()
    if USE_CC:
        sel = nc.dram_tensor("sel", [IDX, 1], i32, kind="ExternalInput").ap()
    o = nc.dram_tensor("o", [T, S], bf16, kind="ExternalOutput").ap()

    groups = [[2 * i, 2 * i + 1] for i in range(N_CORES // 2)]

    with tile.TileContext(nc) as tc:
        with (
            tc.tile_pool(name="const", bufs=1) as cpool,
            tc.tile_pool(name="slab", bufs=3) as slab_pool,
            tc.tile_pool(name="osb", bufs=6) as out_pool,
            tc.tile_pool(name="pj", bufs=2, space="PSUM") as pj_pool,
            tc.tile_pool(name="ps", bufs=6 // PS_BANKS, space="PSUM") as ps_pool,
            tc.tile_pool(name="dram", bufs=1, space="DRAM") as dpool,
        ):
            # --- persistent small tensors ---
            wkqw_sb = cpool.tile([128, DC * W33], bf16, tag="wkqw_sb")
            nc.sync.dma_start(out=wkqw_sb[:], in_=wkqw)
            if USE_CC:
                sel_sb = cpool.tile([IDX, 1], i32, tag="sel_sb")
                nc.sync.dma_start(out=sel_sb[:], in_=sel)

            s33_sb = cpool.tile([W33, T], bf16, tag="s33_sb")
            qT_sb = cpool.tile([IDX, T], bf16, tag="qT_sb")
            ksib_sb = cpool.tile([IDX, T], bf16, tag="ksib_sb")
            w_colb = cpool.tile([128, T // 128], bf16, tag="w_colb")
            w_col = cpool.tile([128, T // 128], f32, tag="w_col")

            if USE_CC:
                kin = dpool.tile([IDX, T], bf16, tag="kin")
                kg = dpool.tile([2 * IDX, T], bf16, tag="kg")

            # --- input slabs (ACT hwdge queue) ---
            slabs = []
            for s in range(n_slabs):
                slab = slab_pool.tile([128, DC * 512], bf16, tag="slab")
                nc.scalar.dma_start(
                    out=slab[:], in_=xh[s * 128:(s + 1) * 128, :])
                slabs.append(slab)

            # --- projections per 512-token group ---
            for g in range(n_slabs):
                slab_v = slabs[g][:].rearrange("p (kd t) -> p kd t", kd=DC)
                pj = pj_pool.tile([W33, 512], f32, tag="pj")
                for kd in range(DC):
                    nc.tensor.matmul(
                        pj[:],
                        wkqw_sb[:, kd * W33:(kd + 1) * W33],
                        slab_v[:, kd, :],
                        start=(kd == 0), stop=(kd == DC - 1),
                    )
                c0, c1 = g * 512, (g + 1) * 512
                if g < NG:
                    nc.vector.tensor_copy(s33_sb[:, c0:c1], pj[:])
                    # engine reads need 32-aligned partition offsets;
                    # DMAs don't — extract q rows and transposed w by DMA
                    nc.gpsimd.dma_start(
                        out=qT_sb[:, c0:c1],
                        in_=s33_sb[IDX:2 * IDX, c0:c1])
                    # w gate transposed column-by-column (ACT hwdge queue)
                    for gi in range(4):
                        t0 = c0 + gi * 128
                        nc.scalar.dma_start(
                            out=w_colb[:, g * 4 + gi:g * 4 + gi + 1],
                            in_=s33_sb[2 * IDX:W33, t0:t0 + 128],
                        )
                    nc.vector.tensor_copy(
                        w_col[:, g * 4:(g + 1) * 4],
                        w_colb[:, g * 4:(g + 1) * 4])
                else:
                    # fallback: other-half keys computed locally
                    nc.vector.tensor_copy(
                        ksib_sb[:, c0 - T:c1 - T], pj[0:IDX, :])

            if USE_CC:
                # one pairwise exchange of the own key rows (64 KB)
                nc.gpsimd.dma_start(out=kin[:], in_=s33_sb[0:IDX, :])
                nc.gpsimd.collective_compute(
                    "AllGather",
                    mybir.AluOpType.bypass,
                    replica_groups=groups,
                    ins=[kin.opt()],
                    outs=[kg.opt()],
                )
                # SPMD-uniform sibling-slot select via index gather
                nc.gpsimd.indirect_dma_start(
                    out=ksib_sb[:],
                    out_offset=None,
                    in_=kg[:],
                    in_offset=bass.IndirectOffsetOnAxis(
                        ap=sel_sb[:, 0:1], axis=0),
                )

            # --- scores + postproc + output, two column phases ---
            # phase 0: own keys (local, no exchange wait)
            # phase 1: sibling keys (after collective + gather)
            TT = T // 128
            gw = PS_BANKS * 512            # postproc group width
            for ph in range(2):
                col0 = ph * 2048
                krhs = s33_sb if ph == 0 else ksib_sb
                for i in range(TT):
                    osb = out_pool.tile([128, 2048], bf16, tag="osb")
                    for cc in range(2048 // gw):
                        ps = ps_pool.tile([128, gw], f32, tag="ps")
                        for jj in range(PS_BANKS):
                            j0 = cc * gw + jj * 512
                            nc.tensor.matmul(
                                ps[:, jj * 512:(jj + 1) * 512],
                                qT_sb[:, i * 128:(i + 1) * 128],
                                krhs[0:IDX, j0:j0 + 512],
                                start=True, stop=True,
                            )
                        oslice = osb[:, cc * gw:(cc + 1) * gw]
                        if FUSED and (i * (2048 // gw) + cc) % 4 == 3:
                            # fused relu+gate on DVE
                            nc.vector.tensor_scalar(
                                out=oslice,
                                in0=ps[:],
                                scalar1=0.0,
                                scalar2=w_col[:, i:i + 1],
                                op0=mybir.AluOpType.max,
                                op1=mybir.AluOpType.mult,
                            )
                        else:
                            nc.scalar.activation(
                                oslice, ps[:],
                                mybir.ActivationFunctionType.Relu,
                            )
                            nc.vector.tensor_scalar_mul(
                                out=oslice,
                                in0=oslice,
                                scalar1=w_col[:, i:i + 1],
                            )
                    nc.sync.dma_start(
                        out=o[i * 128:(i + 1) * 128, col0:col0 + 2048],
                        in_=osb[:],
                    )
    nc.compile()
    _CACHE["nc"] = nc
    return nc


def _make_in_maps(x, Wq, Wk, Ww):
    import ml_dtypes
    bf = ml_dtypes.bfloat16
    w33 = np.concatenate([Wk, Wq, Ww], axis=1).astype(bf)       # [D, 33]
    wkqw = np.ascontiguousarray(
        w33.reshape(DC, 128, W33).transpose(1, 0, 2).reshape(128, DC * W33))
    xbf = x.astype(bf)
    n_slabs = NG if USE_CC else 2 * NG
    in_maps = []
    for c in range(N_CORES):
        b, h = c // 2, c % 2
        own = xbf[b, h * T:(h + 1) * T, :]                       # [T, D]
        if USE_CC:
            xt = own.T                                            # [D, T]
        else:
            oth = xbf[b, (1 - h) * T:(2 - h) * T, :]
            xt = np.concatenate([own, oth], axis=0).T             # [D, S]
        ntok = xt.shape[1]
        xs = np.ascontiguousarray(
            xt.reshape(DC, 128, ntok // 512, 512)
            .transpose(2, 1, 0, 3).reshape(n_slabs * 128, DC * 512))
        im = {"xh": xs, "wkqw": wkqw}
        if USE_CC:
            im["sel"] = ((1 - h) * IDX
                         + np.arange(IDX, dtype=np.int32)).reshape(IDX, 1)
        in_maps.append(im)
    return in_maps


def _assemble(results):
    out = np.empty((B, S, S), dtype=np.float32)
    for c in range(N_CORES):
        b, h = c // 2, c % 2
        oc = np.asarray(results[c]["o"], dtype=np.float32)
        r0 = h * T
        out[b, r0:r0 + T, h * T:(h + 1) * T] = oc[:, 0:T]
        out[b, r0:r0 + T, (1 - h) * T:(2 - h) * T] = oc[:, T:S]
    return out


def kernel(x, Wq, Wk, Ww, _trace_kwargs=None):
    nc = _build_nc()
    in_maps = _make_in_maps(np.asarray(x, dtype=np.float32),
                            np.asarray(Wq, dtype=np.float32),
                            np.asarray(Wk, dtype=np.float32),
                            np.asarray(Ww, dtype=np.float32))
    kw = _trace_kwargs or {}
    res = run_bass_kernel_spmd(nc, in_maps, list(range(N_CORES)), **kw)
    out = _assemble(res.results)
    if _trace_kwargs is not None:
        return out, res
    return out
